# revision 70
# baseline (speedup 1.0000x reference)
"""Trainium2 Bass kernel for DualHazardHead (moe_routing).

Computation per token t:
  x = concat(h, a, d, age)            [594]
  z = gelu(x @ Wt + bt)               [256]
  pw = softmax(h @ Wr + br)           [7]
  inst  = z @ Wbi + bbi + sum_p pw_p (z @ Wei_p + bei_p)   [20]
  group = z @ Wbg + bbg + sum_p pw_p (z @ Weg_p + beg_p)   [20]

Sharding: pure data parallel over B (32 -> 4 per core) on 8 NeuronCores.

Design (per core, NTOK=8192 tokens, 16 macro tiles of 512):
  - x is transposed + tiled on the HOST to [128, 16, 5, 512] bf16 (a
    596th constant-1 feature carries the trunk bias), so each macro is ONE
    fully-contiguous 5 KB/partition DMA: no on-device transposes, no
    PSUM->SBUF copies for x.
  - trunk zT [256, tok] in PSUM (bias via the ones row) -> exact GELU on
    ACT -> bf16 z.
  - router logits pwT [7, tok] on PE; tanh(l/2 + rb/2) computed PHASE-major
    on ACT (router bias fused into the activation), then 4 small PE
    transposes to token-side; softmax exp via (1+t)/(1-t) on DVE so GELU
    and softmax share one ACT table set.
  - heads: one pe PSUM tile [128, 4, 512] spanning 4 banks; columns
    c=(h*20+k)*8+p with p in 0..6 = experts, p=7 = base head; biases are
    PRE-WRITTEN into PSUM by ScalarE (drb) and the z matmuls run with
    start=False, accumulating on top (has_written bits stay set from the
    prewarm / previous macro).  Combine = ONE broadcast multiply by pw8
    (slot 7 = 1.0) + ONE strided reduce over p, both DVE, all 4 banks at
    once (the ~150-cycle DVE op overhead dominates small ops).
  - output is stored partition-major [128, 16, 4, 40] (640 B contiguous
    per partition) on the gpsimd queue; the host unshuffles.  The last
    macro's combine/store is split per-subtile to pipeline the drain.
"""

import os

import numpy as np

B, T = 32, 2048
HID, ACTD, SRC, AGE = 512, 64, 2, 16
TRUNK, BINS, PHASES = 256, 20, 7
IN_DIM = HID + ACTD + SRC + AGE  # 594
NCORES = 8
B_LOC = B // NCORES  # 4
NTOK = B_LOC * T  # 8192
MACRO = 512
NMACRO = NTOK // MACRO  # 16
SUB = MACRO // 128  # 4
NHK = 2 * BINS  # 40 (head, bin) pairs
NP8 = PHASES + 1  # 7 experts + 1 base slot
NCOL = NHK * NP8  # 320 head-matmul output columns
# xT k-block sizes; feature 594 is a constant-1 row carrying the trunk bias
KBLK = [128, 128, 128, 128, 83]

_BUILT = {}
LAST_RESULT = None


def _mm_dt_name():
    return os.environ.get("KERNEL_MM_DT", "bf16")


def _build_module():
    """Build the Bass module (same NEFF for all cores)."""
    import concourse.bass as bass
    import concourse.tile as tile
    from concourse import bacc, mybir
    from concourse.masks import make_identity

    f32 = mybir.dt.float32
    mmdt = {"f32": f32, "f32r": mybir.dt.float32r, "bf16": mybir.dt.bfloat16}[
        _mm_dt_name()
    ]

    AF = mybir.ActivationFunctionType
    ALU = mybir.AluOpType
    ts = bass.ts

    nc = bacc.Bacc("TRN2", target_bir_lowering=False, debug=False)

    # x is host-pre-tiled so each macro's load is one DMA with 5 KB
    # contiguous per partition; the output layout is partition-major so
    # each store writes 640 B contiguous per partition (host unshuffles).
    xt_d = nc.dram_tensor(
        "xt", [128, NMACRO, 5, MACRO], mmdt, kind="ExternalInput"
    )
    wt_d = nc.dram_tensor("wt", [128, 5, TRUNK], mmdt, kind="ExternalInput")
    wr_d = nc.dram_tensor("wr", [128, 4, PHASES], mmdt, kind="ExternalInput")
    wh_d = nc.dram_tensor("wh", [128, 2, NCOL], mmdt, kind="ExternalInput")
    drb_d = nc.dram_tensor("drb", [128, SUB, NCOL], f32, kind="ExternalInput")
    rb2_d = nc.dram_tensor("rb2", [PHASES, 1], f32, kind="ExternalInput")
    out_d = nc.dram_tensor(
        "out", [128, NMACRO, SUB, NHK], f32, kind="ExternalOutput"
    )

    with tile.TileContext(nc) as tc:
        with (
            tc.tile_pool(name="const", bufs=1) as const,
            tc.tile_pool(name="xin", bufs=3) as xin,
            tc.tile_pool(name="zs", bufs=2) as zsp,
            tc.tile_pool(name="sm", bufs=2) as smp,
            tc.tile_pool(name="prod", bufs=3) as prodp,
            tc.tile_pool(name="outp", bufs=2) as outp,
            tc.tile_pool(name="ps_z", bufs=2, space="PSUM") as ps_z,
            tc.tile_pool(name="ps_pw", bufs=1, space="PSUM") as ps_pw,
            tc.tile_pool(name="ps_pt", bufs=1, space="PSUM") as ps_pt,
            tc.tile_pool(name="ps_e", bufs=1, space="PSUM") as ps_e,
        ):
            ident_f = const.tile([128, 128], f32)
            make_identity(nc, ident_f)
            wt = const.tile([128, 5, TRUNK], mmdt)
            nc.gpsimd.dma_start(wt, wt_d[:])
            wr = const.tile([128, 4, PHASES], mmdt)
            nc.gpsimd.dma_start(wr, wr_d[:])
            wh = const.tile([128, 2, NCOL], mmdt)
            nc.gpsimd.dma_start(wh, wh_d[:])
            rb2 = const.tile([PHASES, 1], f32)
            nc.gpsimd.dma_start(rb2, rb2_d[:])
            drb = const.tile([128, SUB, NCOL], f32)
            nc.gpsimd.dma_start(drb, drb_d[:])

            # PE prewarm: consume each const via a cheap dummy matmul so later
            # real PE instructions never stack startup semaphore waits.  The
            # four pe-slot dummies also SET the per-element has_written bits
            # over the full [128, 320] region of every pe PSUM slot, so the
            # steady-state heads matmuls can run with start=False and
            # accumulate onto the ScalarE-prewritten bias (drb).
            pwf = ps_pw.tile([128, MACRO], f32, tag="ppw")
            nc.tensor.transpose(pwf[:7, 0:7], ident_f[:7, :7], ident_f[:7, :7])
            nc.tensor.matmul(
                pwf[:7, 0:128], wr[:, 0, :], wt[:, 0, 0:128],
                start=True, stop=True,
            )
            pe_cur = ps_e.tile([128, SUB, MACRO], f32, tag="pe")
            for _s in range(SUB):
                nc.tensor.matmul(
                    pe_cur[:, _s, 0:NCOL], wt[:, 0, 0:128], wh[:, 0, :],
                    start=True, stop=True,
                )
            # bias pre-write for macro 0 (overwrites the prewarm garbage;
            # has_written bits stay set)
            nc.scalar.copy(out=pe_cur[:, :, 0:NCOL], in_=drb)

            for m in range(NMACRO):
                # ---- load xT tile (host-pre-tiled, fully contiguous) ----
                xt = xin.tile([128, 5, MACRO], mmdt)
                nc.sync.dma_start(xt, xt_d[:, m])

                petile = pe_cur

                # ---- trunk matmuls: zT [256, 512] over 2 PSUM halves ----
                # (trunk bias rides the constant-1 x row in block 4)
                pz0 = ps_z.tile([128, MACRO], f32, tag="pz")
                pz1 = ps_z.tile([128, MACRO], f32, tag="pz")
                for b in range(5):
                    kb = KBLK[b]
                    nc.tensor.matmul(
                        pz0, wt[:kb, b, 0:128], xt[:kb, b, :],
                        start=(b == 0), stop=(b == 4),
                    )
                    nc.tensor.matmul(
                        pz1, wt[:kb, b, 128:256], xt[:kb, b, :],
                        start=(b == 0), stop=(b == 4),
                    )

                # ---- router matmuls: pwT [7, 512] (h = blocks 0..3) ----
                ppw = ps_pw.tile([128, MACRO], f32, tag="ppw")
                for b in range(4):
                    nc.tensor.matmul(
                        ppw[:PHASES], wr[:128, b, :], xt[:128, b, :],
                        start=(b == 0), stop=(b == 3),
                    )

                # ---- GELU (exact; bias already in pz) -> bf16 z ----
                zs = zsp.tile([128, 2, MACRO], mmdt)
                nc.scalar.activation(
                    out=zs[:, 0, :], in_=pz0, func=AF.Gelu, scale=1.0,
                )
                nc.scalar.activation(
                    out=zs[:, 1, :], in_=pz1, func=AF.Gelu, scale=1.0,
                )

                # ---- tanh(l/2 + rb/2) phase-major (router bias fused) ----
                thp = smp.tile([PHASES, MACRO], f32, tag="thp")
                nc.scalar.activation(
                    out=thp, in_=ppw[:PHASES], func=AF.Tanh,
                    bias=rb2, scale=0.5,
                )

                # ---- bias pre-write for the NEXT macro's petile (last in
                # the ACT queue so it never delays gelu/tanh) ----
                if m + 1 < NMACRO:
                    pe_cur = ps_e.tile([128, SUB, MACRO], f32, tag="pe")
                    nc.scalar.copy(out=pe_cur[:, :, 0:NCOL], in_=drb)

                # ---- heads: petile[:, s, 0:320] per subtile ----
                # The z matmuls run with start=False and accumulate onto the
                # ScalarE-prewritten biases (has_written bits stay set from
                # the prewarm / previous macro, so the PE adds instead of
                # overwriting).
                osb = outp.tile([128, SUB, NHK], f32)
                for s in range(SUB):
                    nc.tensor.matmul(
                        petile[:, s, 0:NCOL], zs[:, 0, ts(s, 128)], wh[:, 0, :],
                        start=False, stop=False,
                    )
                    nc.tensor.matmul(
                        petile[:, s, 0:NCOL], zs[:, 1, ts(s, 128)], wh[:, 1, :],
                        start=False, stop=True,
                    )

                # ---- tanh to token-side (after heads in PE queue) ----
                ppt = ps_pt.tile([128, SUB, PHASES], f32, tag="ppt")
                for s in range(SUB):
                    nc.tensor.transpose(
                        ppt[:, s, :], thp[:, ts(s, 128)],
                        ident_f[:PHASES, :PHASES],
                    )

                # ---- softmax from tanh: exp(l) = (1+t)/(1-t), normalize ----
                den = smp.tile([128, SUB, PHASES], f32, tag="den")
                nc.vector.tensor_scalar(
                    out=den, in0=ppt, scalar1=-1.0, scalar2=1.0,
                    op0=ALU.mult, op1=ALU.add,
                )
                pw8 = smp.tile([128, SUB, NP8], f32, tag="pw8")
                nc.gpsimd.memset(pw8[:, :, PHASES : PHASES + 1], 1.0)
                nc.vector.reciprocal_approx_fast(out=den, in_=den)
                nc.vector.scalar_tensor_tensor(
                    out=pw8[:, :, :PHASES], in0=ppt, scalar=1.0, in1=den,
                    op0=ALU.add, op1=ALU.mult,
                )
                ssum = smp.tile([128, SUB], f32, tag="ssum")
                nc.vector.reduce_sum(
                    out=ssum, in_=pw8[:, :, :PHASES], axis=mybir.AxisListType.X
                )
                rec = smp.tile([128, SUB], f32, tag="rec")
                nc.vector.reciprocal_approx_fast(out=rec, in_=ssum)
                nc.vector.tensor_tensor(
                    out=pw8[:, :, :PHASES],
                    in0=pw8[:, :, :PHASES],
                    in1=rec[:, :, None].to_broadcast([128, SUB, PHASES]),
                    op=ALU.mult,
                )

                # ---- combine: one multiply + one reduce over all 4 banks
                # (for the last macro, per-subtile chunks so the final DVE
                # work and the store drain in a pipeline instead of serially)
                prod = prodp.tile([128, SUB, NHK, NP8], mmdt)
                nchunk = SUB if m == NMACRO - 1 else 1
                cs = SUB // nchunk
                for c in range(nchunk):
                    sl = slice(c * cs, (c + 1) * cs)
                    nc.vector.tensor_tensor(
                        out=prod[:, sl],
                        in0=petile[:, sl, 0:NCOL].rearrange(
                            "p s (hk e) -> p s hk e", e=NP8
                        ),
                        in1=pw8[:, sl, None, :].to_broadcast(
                            [128, cs, NHK, NP8]
                        ),
                        op=ALU.mult,
                    )
                    nc.vector.reduce_sum(
                        out=osb[:, sl], in_=prod[:, sl],
                        axis=mybir.AxisListType.X,
                    )
                    # store on the gpsimd queue so it never head-of-line-
                    # blocks the x loads on the sync queue
                    nc.gpsimd.dma_start(out_d[:, m, sl], osb[:, sl])

    nc.compile()
    return nc


def _host_weights(inp):
    """Rearrange weights into on-device layouts (host-side, one-time)."""
    f = np.float32
    wt = np.zeros((128, 5, TRUNK), f)
    for b in range(4):
        wt[:, b, :] = inp["trunk_w"][b * 128 : (b + 1) * 128]
    wt[:82, 4, :] = inp["trunk_w"][512:IN_DIM]
    wt[82, 4, :] = inp["trunk_b"]  # rides the constant-1 x row

    wr = np.zeros((128, 4, PHASES), f)
    for b in range(4):
        wr[:, b, :] = inp["router_w"][b * 128 : (b + 1) * 128]
    rb2 = np.ascontiguousarray(inp["router_b"].reshape(PHASES, 1)) * 0.5

    # heads: col c = (h*20+k)*8 + p ; p<7 experts, p=7 base
    wh_full = np.zeros((TRUNK, NHK, NP8), f)
    dr_full = np.zeros((NHK, NP8), f)
    wh_full[:, :BINS, :PHASES] = np.transpose(inp["inst_exp_w"], (1, 2, 0))
    wh_full[:, BINS:, :PHASES] = np.transpose(inp["group_exp_w"], (1, 2, 0))
    wh_full[:, :BINS, PHASES] = inp["inst_base_w"]
    wh_full[:, BINS:, PHASES] = inp["group_base_w"]
    dr_full[:BINS, :PHASES] = inp["inst_exp_b"].T
    dr_full[BINS:, :PHASES] = inp["group_exp_b"].T
    dr_full[:BINS, PHASES] = inp["inst_base_b"]
    dr_full[BINS:, PHASES] = inp["group_base_b"]
    wh = wh_full.reshape(TRUNK, NCOL).reshape(2, 128, NCOL).transpose(1, 0, 2).copy()
    drb = np.ascontiguousarray(
        np.broadcast_to(dr_full.reshape(1, 1, NCOL), (128, SUB, NCOL))
    )
    return wt, wr, wh, drb, rb2


def kernel(**inputs):
    global LAST_RESULT
    import sys

    if "/opt/trn_rl_repo" not in sys.path:
        sys.path.insert(0, "/opt/trn_rl_repo")
    from concourse.bass_utils import run_bass_kernel_spmd

    inp = {
        k: np.asarray(
            v, dtype=np.float32 if np.asarray(v).dtype != np.int32 else np.int32
        )
        for k, v in inputs.items()
    }

    if "nc" not in _BUILT:
        _BUILT["nc"] = _build_module()
    nc = _BUILT["nc"]

    wt, wr, wh, drb, rb2 = _host_weights(inp)

    x_full = np.concatenate(
        [
            inp["h_t"],
            inp["a_t"],
            inp["d_t"],
            inp["age_embed"],
            np.ones((B, T, 1), np.float32),
        ],
        axis=-1,
    )  # [B, T, 595]; last feature is the constant-1 trunk-bias carrier

    mmdt_name = _mm_dt_name()
    if mmdt_name == "bf16":
        import ml_dtypes

        cdt = ml_dtypes.bfloat16
        x_full = x_full.astype(cdt)
        wt, wr, wh = (a.astype(cdt) for a in (wt, wr, wh))

    in_maps = []
    for c in range(NCORES):
        xc = x_full[c * B_LOC : (c + 1) * B_LOC].reshape(NTOK, IN_DIM + 1)
        # pre-tile to [128, NMACRO, 5, MACRO]: partition p holds feature
        # b*128+p of tokens m*512..m*512+511, 5 KB contiguous per (p, m)
        xpad = np.zeros((640, NTOK), xc.dtype)
        xpad[: IN_DIM + 1] = xc.T
        xtl = np.ascontiguousarray(
            xpad.reshape(5, 128, NMACRO, MACRO).transpose(1, 2, 0, 3)
        )
        in_maps.append(
            {
                "xt": xtl,
                "wt": wt,
                "wr": wr,
                "wh": wh,
                "drb": drb,
                "rb2": rb2,
            }
        )

    res = run_bass_kernel_spmd(nc, in_maps, core_ids=list(range(NCORES)))
    LAST_RESULT = res

    inst = np.empty((B, T, BINS), np.float32)
    grp = np.empty((B, T, BINS), np.float32)
    for c in range(NCORES):
        # device layout [128 p, NMACRO, SUB, NHK] -> token (m, s, p) order
        o = (
            res.results[c]["out"]
            .transpose(1, 2, 0, 3)
            .reshape(B_LOC, T, NHK)
        )
        inst[c * B_LOC : (c + 1) * B_LOC] = o[:, :, :BINS]
        grp[c * B_LOC : (c + 1) * B_LOC] = o[:, :, BINS:]
    return inst, grp


# revision 71
# speedup vs baseline: 1.0141x; 1.0141x over previous
"""Trainium2 Bass kernel for DualHazardHead (moe_routing).

Computation per token t:
  x = concat(h, a, d, age)            [594]
  z = gelu(x @ Wt + bt)               [256]
  pw = softmax(h @ Wr + br)           [7]
  inst  = z @ Wbi + bbi + sum_p pw_p (z @ Wei_p + bei_p)   [20]
  group = z @ Wbg + bbg + sum_p pw_p (z @ Weg_p + beg_p)   [20]

Sharding: pure data parallel over B (32 -> 4 per core) on 8 NeuronCores.

Design (per core, NTOK=8192 tokens, 16 macro tiles of 512):
  - x is transposed + tiled on the HOST to [128, 16, 5, 512] bf16 (a
    596th constant-1 feature carries the trunk bias), so each macro is ONE
    fully-contiguous 5 KB/partition DMA: no on-device transposes, no
    PSUM->SBUF copies for x.
  - trunk zT [256, tok] in PSUM (bias via the ones row) -> exact GELU on
    ACT -> bf16 z.
  - router logits pwT [7, tok] on PE; tanh(l/2 + rb/2) computed PHASE-major
    on ACT (router bias fused into the activation), then 4 small PE
    transposes to token-side; softmax exp via (1+t)/(1-t) on DVE so GELU
    and softmax share one ACT table set.
  - heads: one pe PSUM tile [128, 4, 512] spanning 4 banks; columns
    c=(h*20+k)*8+p with p in 0..6 = experts, p=7 = base head; biases are
    PRE-WRITTEN into PSUM by ScalarE (drb) and the z matmuls run with
    start=False, accumulating on top (has_written bits stay set from the
    prewarm / previous macro).  Combine = ONE broadcast multiply by pw8
    (slot 7 = 1.0) + ONE strided reduce over p, both DVE, all 4 banks at
    once (the ~150-cycle DVE op overhead dominates small ops).
  - output is stored partition-major [128, 16, 4, 40] (640 B contiguous
    per partition) on the gpsimd queue; the host unshuffles.  The last
    macro's combine/store is split per-subtile to pipeline the drain.
"""

import os

import numpy as np

B, T = 32, 2048
HID, ACTD, SRC, AGE = 512, 64, 2, 16
TRUNK, BINS, PHASES = 256, 20, 7
IN_DIM = HID + ACTD + SRC + AGE  # 594
NCORES = 8
B_LOC = B // NCORES  # 4
NTOK = B_LOC * T  # 8192
MACRO = 512
NMACRO = NTOK // MACRO  # 16
SUB = MACRO // 128  # 4
NHK = 2 * BINS  # 40 (head, bin) pairs
NP8 = PHASES + 1  # 7 experts + 1 base slot
NCOL = NHK * NP8  # 320 head-matmul output columns
# xT k-block sizes; feature 594 is a constant-1 row carrying the trunk bias
KBLK = [128, 128, 128, 128, 83]

_BUILT = {}
LAST_RESULT = None


def _mm_dt_name():
    return os.environ.get("KERNEL_MM_DT", "bf16")


def _build_module():
    """Build the Bass module (same NEFF for all cores)."""
    import concourse.bass as bass
    import concourse.tile as tile
    from concourse import bacc, mybir
    from concourse.masks import make_identity

    f32 = mybir.dt.float32
    mmdt = {"f32": f32, "f32r": mybir.dt.float32r, "bf16": mybir.dt.bfloat16}[
        _mm_dt_name()
    ]

    AF = mybir.ActivationFunctionType
    ALU = mybir.AluOpType
    ts = bass.ts

    nc = bacc.Bacc("TRN2", target_bir_lowering=False, debug=False)

    # x is host-pre-tiled so each macro's load is one DMA with 5 KB
    # contiguous per partition; the output layout is partition-major so
    # each store writes 640 B contiguous per partition (host unshuffles).
    xt_d = nc.dram_tensor(
        "xt", [128, NMACRO, 5, MACRO], mmdt, kind="ExternalInput"
    )
    wt_d = nc.dram_tensor("wt", [128, 5, TRUNK], mmdt, kind="ExternalInput")
    wr_d = nc.dram_tensor("wr", [128, 4, PHASES], mmdt, kind="ExternalInput")
    wh_d = nc.dram_tensor("wh", [128, 2, NCOL], mmdt, kind="ExternalInput")
    drb_d = nc.dram_tensor("drb", [128, SUB, NCOL], f32, kind="ExternalInput")
    rb2_d = nc.dram_tensor("rb2", [PHASES, 1], f32, kind="ExternalInput")
    out_d = nc.dram_tensor(
        "out", [128, NMACRO, SUB, NHK], f32, kind="ExternalOutput"
    )

    with tile.TileContext(nc) as tc:
        with (
            tc.tile_pool(name="const", bufs=1) as const,
            tc.tile_pool(name="xin", bufs=3) as xin,
            tc.tile_pool(name="zs", bufs=2) as zsp,
            tc.tile_pool(name="sm", bufs=2) as smp,
            tc.tile_pool(name="prod", bufs=3) as prodp,
            tc.tile_pool(name="outp", bufs=4) as outp,
            tc.tile_pool(name="ps_z", bufs=2, space="PSUM") as ps_z,
            tc.tile_pool(name="ps_pw", bufs=1, space="PSUM") as ps_pw,
            tc.tile_pool(name="ps_pt", bufs=1, space="PSUM") as ps_pt,
            tc.tile_pool(name="ps_e", bufs=1, space="PSUM") as ps_e,
        ):
            ident_f = const.tile([128, 128], f32)
            make_identity(nc, ident_f)
            wt = const.tile([128, 5, TRUNK], mmdt)
            nc.gpsimd.dma_start(wt, wt_d[:])
            wr = const.tile([128, 4, PHASES], mmdt)
            nc.gpsimd.dma_start(wr, wr_d[:])
            wh = const.tile([128, 2, NCOL], mmdt)
            nc.gpsimd.dma_start(wh, wh_d[:])
            rb2 = const.tile([PHASES, 1], f32)
            nc.gpsimd.dma_start(rb2, rb2_d[:])
            drb = const.tile([128, SUB, NCOL], f32)
            nc.gpsimd.dma_start(drb, drb_d[:])

            # PE prewarm: consume each const via a cheap dummy matmul so later
            # real PE instructions never stack startup semaphore waits.  The
            # four pe-slot dummies also SET the per-element has_written bits
            # over the full [128, 320] region of every pe PSUM slot, so the
            # steady-state heads matmuls can run with start=False and
            # accumulate onto the ScalarE-prewritten bias (drb).
            pwf = ps_pw.tile([128, MACRO], f32, tag="ppw")
            nc.tensor.transpose(pwf[:7, 0:7], ident_f[:7, :7], ident_f[:7, :7])
            nc.tensor.matmul(
                pwf[:7, 0:128], wr[:, 0, :], wt[:, 0, 0:128],
                start=True, stop=True,
            )
            pe_cur = ps_e.tile([128, SUB, MACRO], f32, tag="pe")
            for _s in range(SUB):
                nc.tensor.matmul(
                    pe_cur[:, _s, 0:NCOL], wt[:, 0, 0:128], wh[:, 0, :],
                    start=True, stop=True,
                )
            # bias pre-write for macro 0 (overwrites the prewarm garbage;
            # has_written bits stay set)
            nc.scalar.copy(out=pe_cur[:, :, 0:NCOL], in_=drb)

            for m in range(NMACRO):
                # ---- load xT tile (host-pre-tiled, fully contiguous) ----
                xt = xin.tile([128, 5, MACRO], mmdt)
                nc.sync.dma_start(xt, xt_d[:, m])

                petile = pe_cur

                # ---- trunk matmuls: zT [256, 512] over 2 PSUM halves ----
                # (trunk bias rides the constant-1 x row in block 4)
                pz0 = ps_z.tile([128, MACRO], f32, tag="pz")
                pz1 = ps_z.tile([128, MACRO], f32, tag="pz")
                for b in range(5):
                    kb = KBLK[b]
                    nc.tensor.matmul(
                        pz0, wt[:kb, b, 0:128], xt[:kb, b, :],
                        start=(b == 0), stop=(b == 4),
                    )
                    nc.tensor.matmul(
                        pz1, wt[:kb, b, 128:256], xt[:kb, b, :],
                        start=(b == 0), stop=(b == 4),
                    )

                # ---- router matmuls: pwT [7, 512] (h = blocks 0..3) ----
                ppw = ps_pw.tile([128, MACRO], f32, tag="ppw")
                for b in range(4):
                    nc.tensor.matmul(
                        ppw[:PHASES], wr[:128, b, :], xt[:128, b, :],
                        start=(b == 0), stop=(b == 3),
                    )

                # ---- GELU (exact; bias already in pz) -> bf16 z ----
                zs = zsp.tile([128, 2, MACRO], mmdt)
                nc.scalar.activation(
                    out=zs[:, 0, :], in_=pz0, func=AF.Gelu, scale=1.0,
                )
                nc.scalar.activation(
                    out=zs[:, 1, :], in_=pz1, func=AF.Gelu, scale=1.0,
                )

                # ---- tanh(l/2 + rb/2) phase-major (router bias fused) ----
                thp = smp.tile([PHASES, MACRO], f32, tag="thp")
                nc.scalar.activation(
                    out=thp, in_=ppw[:PHASES], func=AF.Tanh,
                    bias=rb2, scale=0.5,
                )

                # ---- bias pre-write for the NEXT macro's petile (last in
                # the ACT queue so it never delays gelu/tanh) ----
                if m + 1 < NMACRO:
                    pe_cur = ps_e.tile([128, SUB, MACRO], f32, tag="pe")
                    nc.scalar.copy(out=pe_cur[:, :, 0:NCOL], in_=drb)

                # ---- heads: petile[:, s, 0:320] per subtile ----
                # The z matmuls run with start=False and accumulate onto the
                # ScalarE-prewritten biases (has_written bits stay set from
                # the prewarm / previous macro, so the PE adds instead of
                # overwriting).
                osb = outp.tile([128, SUB, NHK], f32)
                for s in range(SUB):
                    nc.tensor.matmul(
                        petile[:, s, 0:NCOL], zs[:, 0, ts(s, 128)], wh[:, 0, :],
                        start=False, stop=False,
                    )
                    nc.tensor.matmul(
                        petile[:, s, 0:NCOL], zs[:, 1, ts(s, 128)], wh[:, 1, :],
                        start=False, stop=True,
                    )

                # ---- tanh to token-side (after heads in PE queue) ----
                ppt = ps_pt.tile([128, SUB, PHASES], f32, tag="ppt")
                for s in range(SUB):
                    nc.tensor.transpose(
                        ppt[:, s, :], thp[:, ts(s, 128)],
                        ident_f[:PHASES, :PHASES],
                    )

                # ---- softmax from tanh: exp(l) = (1+t)/(1-t), normalize ----
                den = smp.tile([128, SUB, PHASES], f32, tag="den")
                nc.vector.tensor_scalar(
                    out=den, in0=ppt, scalar1=-1.0, scalar2=1.0,
                    op0=ALU.mult, op1=ALU.add,
                )
                pw8 = smp.tile([128, SUB, NP8], f32, tag="pw8")
                nc.gpsimd.memset(pw8[:, :, PHASES : PHASES + 1], 1.0)
                nc.vector.reciprocal_approx_fast(out=den, in_=den)
                nc.vector.scalar_tensor_tensor(
                    out=pw8[:, :, :PHASES], in0=ppt, scalar=1.0, in1=den,
                    op0=ALU.add, op1=ALU.mult,
                )
                ssum = smp.tile([128, SUB], f32, tag="ssum")
                nc.vector.reduce_sum(
                    out=ssum, in_=pw8[:, :, :PHASES], axis=mybir.AxisListType.X
                )
                rec = smp.tile([128, SUB], f32, tag="rec")
                nc.vector.reciprocal_approx_fast(out=rec, in_=ssum)
                nc.vector.tensor_tensor(
                    out=pw8[:, :, :PHASES],
                    in0=pw8[:, :, :PHASES],
                    in1=rec[:, :, None].to_broadcast([128, SUB, PHASES]),
                    op=ALU.mult,
                )

                # ---- combine: one multiply + one reduce over all 4 banks
                # (for the last macro, per-subtile chunks so the final DVE
                # work and the store drain in a pipeline instead of serially)
                prod = prodp.tile([128, SUB, NHK, NP8], mmdt)
                nchunk = SUB if m == NMACRO - 1 else 1
                cs = SUB // nchunk
                for c in range(nchunk):
                    sl = slice(c * cs, (c + 1) * cs)
                    nc.vector.tensor_tensor(
                        out=prod[:, sl],
                        in0=petile[:, sl, 0:NCOL].rearrange(
                            "p s (hk e) -> p s hk e", e=NP8
                        ),
                        in1=pw8[:, sl, None, :].to_broadcast(
                            [128, cs, NHK, NP8]
                        ),
                        op=ALU.mult,
                    )
                    nc.vector.reduce_sum(
                        out=osb[:, sl], in_=prod[:, sl],
                        axis=mybir.AxisListType.X,
                    )
                    # store on the gpsimd queue so it never head-of-line-
                    # blocks the x loads on the sync queue
                    nc.gpsimd.dma_start(out_d[:, m, sl], osb[:, sl])

    nc.compile()
    return nc


def _host_weights(inp):
    """Rearrange weights into on-device layouts (host-side, one-time)."""
    f = np.float32
    wt = np.zeros((128, 5, TRUNK), f)
    for b in range(4):
        wt[:, b, :] = inp["trunk_w"][b * 128 : (b + 1) * 128]
    wt[:82, 4, :] = inp["trunk_w"][512:IN_DIM]
    wt[82, 4, :] = inp["trunk_b"]  # rides the constant-1 x row

    wr = np.zeros((128, 4, PHASES), f)
    for b in range(4):
        wr[:, b, :] = inp["router_w"][b * 128 : (b + 1) * 128]
    rb2 = np.ascontiguousarray(inp["router_b"].reshape(PHASES, 1)) * 0.5

    # heads: col c = (h*20+k)*8 + p ; p<7 experts, p=7 base
    wh_full = np.zeros((TRUNK, NHK, NP8), f)
    dr_full = np.zeros((NHK, NP8), f)
    wh_full[:, :BINS, :PHASES] = np.transpose(inp["inst_exp_w"], (1, 2, 0))
    wh_full[:, BINS:, :PHASES] = np.transpose(inp["group_exp_w"], (1, 2, 0))
    wh_full[:, :BINS, PHASES] = inp["inst_base_w"]
    wh_full[:, BINS:, PHASES] = inp["group_base_w"]
    dr_full[:BINS, :PHASES] = inp["inst_exp_b"].T
    dr_full[BINS:, :PHASES] = inp["group_exp_b"].T
    dr_full[:BINS, PHASES] = inp["inst_base_b"]
    dr_full[BINS:, PHASES] = inp["group_base_b"]
    wh = wh_full.reshape(TRUNK, NCOL).reshape(2, 128, NCOL).transpose(1, 0, 2).copy()
    drb = np.ascontiguousarray(
        np.broadcast_to(dr_full.reshape(1, 1, NCOL), (128, SUB, NCOL))
    )
    return wt, wr, wh, drb, rb2


def kernel(**inputs):
    global LAST_RESULT
    import sys

    if "/opt/trn_rl_repo" not in sys.path:
        sys.path.insert(0, "/opt/trn_rl_repo")
    from concourse.bass_utils import run_bass_kernel_spmd

    inp = {
        k: np.asarray(
            v, dtype=np.float32 if np.asarray(v).dtype != np.int32 else np.int32
        )
        for k, v in inputs.items()
    }

    if "nc" not in _BUILT:
        _BUILT["nc"] = _build_module()
    nc = _BUILT["nc"]

    wt, wr, wh, drb, rb2 = _host_weights(inp)

    x_full = np.concatenate(
        [
            inp["h_t"],
            inp["a_t"],
            inp["d_t"],
            inp["age_embed"],
            np.ones((B, T, 1), np.float32),
        ],
        axis=-1,
    )  # [B, T, 595]; last feature is the constant-1 trunk-bias carrier

    mmdt_name = _mm_dt_name()
    if mmdt_name == "bf16":
        import ml_dtypes

        cdt = ml_dtypes.bfloat16
        x_full = x_full.astype(cdt)
        wt, wr, wh = (a.astype(cdt) for a in (wt, wr, wh))

    in_maps = []
    for c in range(NCORES):
        xc = x_full[c * B_LOC : (c + 1) * B_LOC].reshape(NTOK, IN_DIM + 1)
        # pre-tile to [128, NMACRO, 5, MACRO]: partition p holds feature
        # b*128+p of tokens m*512..m*512+511, 5 KB contiguous per (p, m)
        xpad = np.zeros((640, NTOK), xc.dtype)
        xpad[: IN_DIM + 1] = xc.T
        xtl = np.ascontiguousarray(
            xpad.reshape(5, 128, NMACRO, MACRO).transpose(1, 2, 0, 3)
        )
        in_maps.append(
            {
                "xt": xtl,
                "wt": wt,
                "wr": wr,
                "wh": wh,
                "drb": drb,
                "rb2": rb2,
            }
        )

    res = run_bass_kernel_spmd(nc, in_maps, core_ids=list(range(NCORES)))
    LAST_RESULT = res

    inst = np.empty((B, T, BINS), np.float32)
    grp = np.empty((B, T, BINS), np.float32)
    for c in range(NCORES):
        # device layout [128 p, NMACRO, SUB, NHK] -> token (m, s, p) order
        o = (
            res.results[c]["out"]
            .transpose(1, 2, 0, 3)
            .reshape(B_LOC, T, NHK)
        )
        inst[c * B_LOC : (c + 1) * B_LOC] = o[:, :, :BINS]
        grp[c * B_LOC : (c + 1) * B_LOC] = o[:, :, BINS:]
    return inst, grp


# revision 77
# speedup vs baseline: 1.0312x; 1.0168x over previous
"""Trainium2 Bass kernel for DualHazardHead (moe_routing).

Computation per token t:
  x = concat(h, a, d, age)            [594]
  z = gelu(x @ Wt + bt)               [256]
  pw = softmax(h @ Wr + br)           [7]
  inst  = z @ Wbi + bbi + sum_p pw_p (z @ Wei_p + bei_p)   [20]
  group = z @ Wbg + bbg + sum_p pw_p (z @ Weg_p + beg_p)   [20]

Sharding: pure data parallel over B (32 -> 4 per core) on 8 NeuronCores.

Design (per core, NTOK=8192 tokens, 16 macro tiles of 512):
  - x is transposed + tiled on the HOST to [128, 16, 5, 512] bf16 (a
    596th constant-1 feature carries the trunk bias), so each macro is ONE
    fully-contiguous 5 KB/partition DMA: no on-device transposes, no
    PSUM->SBUF copies for x.
  - trunk zT [256, tok] in PSUM (bias via the ones row) -> exact GELU on
    ACT -> bf16 z.
  - router logits pwT [7, tok] on PE; tanh(l/2 + rb/2) computed PHASE-major
    on ACT (router bias fused into the activation), then 4 small PE
    transposes to token-side; softmax exp via (1+t)/(1-t) on DVE so GELU
    and softmax share one ACT table set.
  - heads: one pe PSUM tile [128, 4, 512] spanning 4 banks; columns
    c=(h*20+k)*8+p with p in 0..6 = experts, p=7 = base head; biases are
    PRE-WRITTEN into PSUM by ScalarE (drb) and the z matmuls run with
    start=False, accumulating on top (has_written bits stay set from the
    prewarm / previous macro).  Combine = ONE broadcast multiply by pw8
    (slot 7 = 1.0) + ONE strided reduce over p, both DVE, all 4 banks at
    once (the ~150-cycle DVE op overhead dominates small ops).
  - output is stored partition-major [128, 16, 4, 40] (640 B contiguous
    per partition) on the gpsimd queue; the host unshuffles.  The last
    macro's combine/store is split per-subtile to pipeline the drain.
"""

import os

import numpy as np

B, T = 32, 2048
HID, ACTD, SRC, AGE = 512, 64, 2, 16
TRUNK, BINS, PHASES = 256, 20, 7
IN_DIM = HID + ACTD + SRC + AGE  # 594
NCORES = 8
B_LOC = B // NCORES  # 4
NTOK = B_LOC * T  # 8192
MACRO = 512
NMACRO = NTOK // MACRO  # 16
SUB = MACRO // 128  # 4
NHK = 2 * BINS  # 40 (head, bin) pairs
NP8 = PHASES + 1  # 7 experts + 1 base slot
NCOL = NHK * NP8  # 320 head-matmul output columns
# xT k-block sizes; feature 594 is a constant-1 row carrying the trunk bias
KBLK = [128, 128, 128, 128, 83]

_BUILT = {}
LAST_RESULT = None


def _mm_dt_name():
    return os.environ.get("KERNEL_MM_DT", "bf16")


def _build_module():
    """Build the Bass module (same NEFF for all cores)."""
    import concourse.bass as bass
    import concourse.tile as tile
    from concourse import bacc, mybir
    from concourse.masks import make_identity

    f32 = mybir.dt.float32
    mmdt = {"f32": f32, "f32r": mybir.dt.float32r, "bf16": mybir.dt.bfloat16}[
        _mm_dt_name()
    ]

    AF = mybir.ActivationFunctionType
    ALU = mybir.AluOpType
    ts = bass.ts

    nc = bacc.Bacc("TRN2", target_bir_lowering=False, debug=False)

    # x is host-pre-tiled so each macro's load is one DMA with 5 KB
    # contiguous per partition; the output layout is partition-major so
    # each store writes 640 B contiguous per partition (host unshuffles).
    xt_d = nc.dram_tensor(
        "xt", [128, NMACRO, 5, MACRO], mmdt, kind="ExternalInput"
    )
    wt_d = nc.dram_tensor("wt", [128, 5, TRUNK], mmdt, kind="ExternalInput")
    wr_d = nc.dram_tensor("wr", [128, 4, PHASES], mmdt, kind="ExternalInput")
    wh_d = nc.dram_tensor("wh", [128, 2, NCOL], mmdt, kind="ExternalInput")
    drb_d = nc.dram_tensor("drb", [128, NCOL], f32, kind="ExternalInput")
    rb2_d = nc.dram_tensor("rb2", [PHASES, 1], f32, kind="ExternalInput")
    out_d = nc.dram_tensor(
        "out", [128, NMACRO, SUB, NHK], f32, kind="ExternalOutput"
    )

    with tile.TileContext(nc) as tc:
        with (
            tc.tile_pool(name="const", bufs=1) as const,
            tc.tile_pool(name="xin", bufs=3) as xin,
            tc.tile_pool(name="zs", bufs=2) as zsp,
            tc.tile_pool(name="sm", bufs=2) as smp,
            tc.tile_pool(name="prod", bufs=3) as prodp,
            tc.tile_pool(name="outp", bufs=4) as outp,
            tc.tile_pool(name="ps_z", bufs=2, space="PSUM") as ps_z,
            tc.tile_pool(name="ps_pw", bufs=1, space="PSUM") as ps_pw,
            tc.tile_pool(name="ps_pt", bufs=1, space="PSUM") as ps_pt,
            tc.tile_pool(name="ps_e", bufs=1, space="PSUM") as ps_e,
        ):
            ident_f = const.tile([128, 128], f32)
            make_identity(nc, ident_f)
            wt = const.tile([128, 5, TRUNK], mmdt)
            nc.gpsimd.dma_start(wt, wt_d[:])
            wr = const.tile([128, 4, PHASES], mmdt)
            nc.gpsimd.dma_start(wr, wr_d[:])
            wh = const.tile([128, 2, NCOL], mmdt)
            nc.gpsimd.dma_start(wh, wh_d[:])
            rb2 = const.tile([PHASES, 1], f32)
            nc.gpsimd.dma_start(rb2, rb2_d[:])
            drb = const.tile([128, NCOL], f32)
            nc.gpsimd.dma_start(drb, drb_d[:])

            # PE prewarm is minimal (ident only): trunk(0) then gates solely
            # on the wt + xt(0) DMAs.  The four pe-slot has_written-setting
            # dummies are emitted inside macro 0, after the router, so they
            # do not put the wh DMA on the critical path of the first
            # matmul.
            pwf = ps_pw.tile([128, MACRO], f32, tag="ppw")
            nc.tensor.transpose(pwf[:7, 0:7], ident_f[:7, :7], ident_f[:7, :7])
            pe_cur = ps_e.tile([128, SUB, MACRO], f32, tag="pe")

            for m in range(NMACRO):
                # ---- load xT tile (host-pre-tiled, fully contiguous) ----
                xt = xin.tile([128, 5, MACRO], mmdt)
                nc.sync.dma_start(xt, xt_d[:, m])

                petile = pe_cur

                # ---- trunk matmuls: zT [256, 512] over 2 PSUM halves ----
                # (trunk bias rides the constant-1 x row in block 4)
                pz0 = ps_z.tile([128, MACRO], f32, tag="pz")
                pz1 = ps_z.tile([128, MACRO], f32, tag="pz")
                for b in range(5):
                    kb = KBLK[b]
                    nc.tensor.matmul(
                        pz0, wt[:kb, b, 0:128], xt[:kb, b, :],
                        start=(b == 0), stop=(b == 4),
                    )
                    nc.tensor.matmul(
                        pz1, wt[:kb, b, 128:256], xt[:kb, b, :],
                        start=(b == 0), stop=(b == 4),
                    )

                # ---- router matmuls: pwT [7, 512] (h = blocks 0..3) ----
                ppw = ps_pw.tile([128, MACRO], f32, tag="ppw")
                for b in range(4):
                    nc.tensor.matmul(
                        ppw[:PHASES], wr[:128, b, :], xt[:128, b, :],
                        start=(b == 0), stop=(b == 3),
                    )

                if m == 0:
                    # set has_written over the full [128, 320] region of
                    # every pe PSUM slot so the steady-state heads matmuls
                    # can run with start=False and accumulate onto the
                    # ScalarE-prewritten bias
                    for _s in range(SUB):
                        nc.tensor.matmul(
                            petile[:, _s, 0:NCOL], wt[:, 0, 0:128],
                            wh[:, 0, :], start=True, stop=True,
                        )

                # ---- GELU (exact; bias already in pz) -> bf16 z ----
                zs = zsp.tile([128, 2, MACRO], mmdt)
                nc.scalar.activation(
                    out=zs[:, 0, :], in_=pz0, func=AF.Gelu, scale=1.0,
                )
                nc.scalar.activation(
                    out=zs[:, 1, :], in_=pz1, func=AF.Gelu, scale=1.0,
                )
                if m == 0:
                    # bias pre-write for macro 0 (after the gelus in the
                    # ACT queue; overwrites the prewarm garbage, has_written
                    # bits stay set)
                    nc.scalar.copy(
                        out=petile[:, :, 0:NCOL],
                        in_=drb[:, None, :].to_broadcast([128, SUB, NCOL]),
                    )

                # ---- tanh(l/2 + rb/2) phase-major (router bias fused) ----
                thp = smp.tile([PHASES, MACRO], f32, tag="thp")
                nc.scalar.activation(
                    out=thp, in_=ppw[:PHASES], func=AF.Tanh,
                    bias=rb2, scale=0.5,
                )

                # ---- bias pre-write for the NEXT macro's petile (last in
                # the ACT queue so it never delays gelu/tanh) ----
                if m + 1 < NMACRO:
                    pe_cur = ps_e.tile([128, SUB, MACRO], f32, tag="pe")
                    nc.scalar.copy(
                        out=pe_cur[:, :, 0:NCOL],
                        in_=drb[:, None, :].to_broadcast([128, SUB, NCOL]),
                    )

                # ---- heads: petile[:, s, 0:320] per subtile ----
                # The z matmuls run with start=False and accumulate onto the
                # ScalarE-prewritten biases (has_written bits stay set from
                # the prewarm / previous macro, so the PE adds instead of
                # overwriting).
                osb = outp.tile([128, SUB, NHK], f32)
                for s in range(SUB):
                    nc.tensor.matmul(
                        petile[:, s, 0:NCOL], zs[:, 0, ts(s, 128)], wh[:, 0, :],
                        start=False, stop=False,
                    )
                    nc.tensor.matmul(
                        petile[:, s, 0:NCOL], zs[:, 1, ts(s, 128)], wh[:, 1, :],
                        start=False, stop=True,
                    )

                # ---- tanh to token-side (after heads in PE queue) ----
                ppt = ps_pt.tile([128, SUB, PHASES], f32, tag="ppt")
                for s in range(SUB):
                    nc.tensor.transpose(
                        ppt[:, s, :], thp[:, ts(s, 128)],
                        ident_f[:PHASES, :PHASES],
                    )

                # ---- softmax from tanh: exp(l) = (1+t)/(1-t), normalize ----
                den = smp.tile([128, SUB, PHASES], f32, tag="den")
                nc.vector.tensor_scalar(
                    out=den, in0=ppt, scalar1=-1.0, scalar2=1.0,
                    op0=ALU.mult, op1=ALU.add,
                )
                pw8 = smp.tile([128, SUB, NP8], f32, tag="pw8")
                nc.gpsimd.memset(pw8[:, :, PHASES : PHASES + 1], 1.0)
                nc.vector.reciprocal_approx_fast(out=den, in_=den)
                nc.vector.scalar_tensor_tensor(
                    out=pw8[:, :, :PHASES], in0=ppt, scalar=1.0, in1=den,
                    op0=ALU.add, op1=ALU.mult,
                )
                ssum = smp.tile([128, SUB], f32, tag="ssum")
                nc.vector.reduce_sum(
                    out=ssum, in_=pw8[:, :, :PHASES], axis=mybir.AxisListType.X
                )
                rec = smp.tile([128, SUB], f32, tag="rec")
                nc.vector.reciprocal_approx_fast(out=rec, in_=ssum)
                nc.vector.tensor_tensor(
                    out=pw8[:, :, :PHASES],
                    in0=pw8[:, :, :PHASES],
                    in1=rec[:, :, None].to_broadcast([128, SUB, PHASES]),
                    op=ALU.mult,
                )

                # ---- combine: one multiply + one reduce over all 4 banks
                # (for the last macro, per-subtile chunks so the final DVE
                # work and the store drain in a pipeline instead of serially)
                prod = prodp.tile([128, SUB, NHK, NP8], mmdt)
                nchunk = SUB if m == NMACRO - 1 else 1
                cs = SUB // nchunk
                for c in range(nchunk):
                    sl = slice(c * cs, (c + 1) * cs)
                    nc.vector.tensor_tensor(
                        out=prod[:, sl],
                        in0=petile[:, sl, 0:NCOL].rearrange(
                            "p s (hk e) -> p s hk e", e=NP8
                        ),
                        in1=pw8[:, sl, None, :].to_broadcast(
                            [128, cs, NHK, NP8]
                        ),
                        op=ALU.mult,
                    )
                    nc.vector.reduce_sum(
                        out=osb[:, sl], in_=prod[:, sl],
                        axis=mybir.AxisListType.X,
                    )
                    # store on the gpsimd queue so it never head-of-line-
                    # blocks the x loads on the sync queue
                    nc.gpsimd.dma_start(out_d[:, m, sl], osb[:, sl])

    nc.compile()
    return nc


def _host_weights(inp):
    """Rearrange weights into on-device layouts (host-side, one-time)."""
    f = np.float32
    wt = np.zeros((128, 5, TRUNK), f)
    for b in range(4):
        wt[:, b, :] = inp["trunk_w"][b * 128 : (b + 1) * 128]
    wt[:82, 4, :] = inp["trunk_w"][512:IN_DIM]
    wt[82, 4, :] = inp["trunk_b"]  # rides the constant-1 x row

    wr = np.zeros((128, 4, PHASES), f)
    for b in range(4):
        wr[:, b, :] = inp["router_w"][b * 128 : (b + 1) * 128]
    rb2 = np.ascontiguousarray(inp["router_b"].reshape(PHASES, 1)) * 0.5

    # heads: col c = (h*20+k)*8 + p ; p<7 experts, p=7 base
    wh_full = np.zeros((TRUNK, NHK, NP8), f)
    dr_full = np.zeros((NHK, NP8), f)
    wh_full[:, :BINS, :PHASES] = np.transpose(inp["inst_exp_w"], (1, 2, 0))
    wh_full[:, BINS:, :PHASES] = np.transpose(inp["group_exp_w"], (1, 2, 0))
    wh_full[:, :BINS, PHASES] = inp["inst_base_w"]
    wh_full[:, BINS:, PHASES] = inp["group_base_w"]
    dr_full[:BINS, :PHASES] = inp["inst_exp_b"].T
    dr_full[BINS:, :PHASES] = inp["group_exp_b"].T
    dr_full[:BINS, PHASES] = inp["inst_base_b"]
    dr_full[BINS:, PHASES] = inp["group_base_b"]
    wh = wh_full.reshape(TRUNK, NCOL).reshape(2, 128, NCOL).transpose(1, 0, 2).copy()
    drb = np.ascontiguousarray(
        np.broadcast_to(dr_full.reshape(1, NCOL), (128, NCOL))
    )
    return wt, wr, wh, drb, rb2


def kernel(**inputs):
    global LAST_RESULT
    import sys

    if "/opt/trn_rl_repo" not in sys.path:
        sys.path.insert(0, "/opt/trn_rl_repo")
    from concourse.bass_utils import run_bass_kernel_spmd

    inp = {
        k: np.asarray(
            v, dtype=np.float32 if np.asarray(v).dtype != np.int32 else np.int32
        )
        for k, v in inputs.items()
    }

    if "nc" not in _BUILT:
        _BUILT["nc"] = _build_module()
    nc = _BUILT["nc"]

    wt, wr, wh, drb, rb2 = _host_weights(inp)

    x_full = np.concatenate(
        [
            inp["h_t"],
            inp["a_t"],
            inp["d_t"],
            inp["age_embed"],
            np.ones((B, T, 1), np.float32),
        ],
        axis=-1,
    )  # [B, T, 595]; last feature is the constant-1 trunk-bias carrier

    mmdt_name = _mm_dt_name()
    if mmdt_name == "bf16":
        import ml_dtypes

        cdt = ml_dtypes.bfloat16
        x_full = x_full.astype(cdt)
        wt, wr, wh = (a.astype(cdt) for a in (wt, wr, wh))

    in_maps = []
    for c in range(NCORES):
        xc = x_full[c * B_LOC : (c + 1) * B_LOC].reshape(NTOK, IN_DIM + 1)
        # pre-tile to [128, NMACRO, 5, MACRO]: partition p holds feature
        # b*128+p of tokens m*512..m*512+511, 5 KB contiguous per (p, m)
        xpad = np.zeros((640, NTOK), xc.dtype)
        xpad[: IN_DIM + 1] = xc.T
        xtl = np.ascontiguousarray(
            xpad.reshape(5, 128, NMACRO, MACRO).transpose(1, 2, 0, 3)
        )
        in_maps.append(
            {
                "xt": xtl,
                "wt": wt,
                "wr": wr,
                "wh": wh,
                "drb": drb,
                "rb2": rb2,
            }
        )

    res = run_bass_kernel_spmd(nc, in_maps, core_ids=list(range(NCORES)))
    LAST_RESULT = res

    inst = np.empty((B, T, BINS), np.float32)
    grp = np.empty((B, T, BINS), np.float32)
    for c in range(NCORES):
        # device layout [128 p, NMACRO, SUB, NHK] -> token (m, s, p) order
        o = (
            res.results[c]["out"]
            .transpose(1, 2, 0, 3)
            .reshape(B_LOC, T, NHK)
        )
        inst[c * B_LOC : (c + 1) * B_LOC] = o[:, :, :BINS]
        grp[c * B_LOC : (c + 1) * B_LOC] = o[:, :, BINS:]
    return inst, grp


# revision 78
# speedup vs baseline: 1.0591x; 1.0270x over previous
"""Trainium2 Bass kernel for DualHazardHead (moe_routing).

Computation per token t:
  x = concat(h, a, d, age)            [594]
  z = gelu(x @ Wt + bt)               [256]
  pw = softmax(h @ Wr + br)           [7]
  inst  = z @ Wbi + bbi + sum_p pw_p (z @ Wei_p + bei_p)   [20]
  group = z @ Wbg + bbg + sum_p pw_p (z @ Weg_p + beg_p)   [20]

Sharding: pure data parallel over B (32 -> 4 per core) on 8 NeuronCores.

Design (per core, NTOK=8192 tokens, 16 macro tiles of 512):
  - x is transposed + tiled on the HOST to [128, 16, 5, 512] bf16 (a
    596th constant-1 feature carries the trunk bias), so each macro is ONE
    fully-contiguous 5 KB/partition DMA: no on-device transposes, no
    PSUM->SBUF copies for x.
  - trunk zT [256, tok] in PSUM (bias via the ones row) -> exact GELU on
    ACT -> bf16 z.
  - router logits pwT [7, tok] on PE; tanh(l/2 + rb/2) computed PHASE-major
    on ACT (router bias fused into the activation), then 4 small PE
    transposes to token-side; softmax exp via (1+t)/(1-t) on DVE so GELU
    and softmax share one ACT table set.
  - heads: one pe PSUM tile [128, 4, 512] spanning 4 banks; columns
    c=(h*20+k)*8+p with p in 0..6 = experts, p=7 = base head; biases are
    PRE-WRITTEN into PSUM by ScalarE (drb) and the z matmuls run with
    start=False, accumulating on top (has_written bits stay set from the
    prewarm / previous macro).  Combine = ONE broadcast multiply by pw8
    (slot 7 = 1.0) + ONE strided reduce over p, both DVE, all 4 banks at
    once (the ~150-cycle DVE op overhead dominates small ops).
  - output is stored partition-major [128, 16, 4, 40] (640 B contiguous
    per partition) on the gpsimd queue; the host unshuffles.  The last
    macro's combine/store is split per-subtile to pipeline the drain.
"""

import os

import numpy as np

B, T = 32, 2048
HID, ACTD, SRC, AGE = 512, 64, 2, 16
TRUNK, BINS, PHASES = 256, 20, 7
IN_DIM = HID + ACTD + SRC + AGE  # 594
NCORES = 8
B_LOC = B // NCORES  # 4
NTOK = B_LOC * T  # 8192
MACRO = 512
NMACRO = NTOK // MACRO  # 16
SUB = MACRO // 128  # 4
NHK = 2 * BINS  # 40 (head, bin) pairs
NP8 = PHASES + 1  # 7 experts + 1 base slot
NCOL = NHK * NP8  # 320 head-matmul output columns
# xT k-block sizes; feature 594 is a constant-1 row carrying the trunk bias
KBLK = [128, 128, 128, 128, 83]

_BUILT = {}
LAST_RESULT = None


def _mm_dt_name():
    return os.environ.get("KERNEL_MM_DT", "bf16")


def _build_module():
    """Build the Bass module (same NEFF for all cores)."""
    import concourse.bass as bass
    import concourse.tile as tile
    from concourse import bacc, mybir
    from concourse.masks import make_identity

    f32 = mybir.dt.float32
    mmdt = {"f32": f32, "f32r": mybir.dt.float32r, "bf16": mybir.dt.bfloat16}[
        _mm_dt_name()
    ]

    AF = mybir.ActivationFunctionType
    ALU = mybir.AluOpType
    ts = bass.ts

    nc = bacc.Bacc("TRN2", target_bir_lowering=False, debug=False)

    # x is host-pre-tiled so each macro's load is one DMA with 5 KB
    # contiguous per partition; the output layout is partition-major so
    # each store writes 640 B contiguous per partition (host unshuffles).
    xt_d = nc.dram_tensor(
        "xt", [128, NMACRO, 5, MACRO], mmdt, kind="ExternalInput"
    )
    wt_d = nc.dram_tensor("wt", [128, 5, TRUNK], mmdt, kind="ExternalInput")
    wr_d = nc.dram_tensor("wr", [128, 4, PHASES], mmdt, kind="ExternalInput")
    wh_d = nc.dram_tensor("wh", [128, 2, NCOL], mmdt, kind="ExternalInput")
    drb_d = nc.dram_tensor("drb", [128, NCOL], f32, kind="ExternalInput")
    rb2_d = nc.dram_tensor("rb2", [PHASES, 1], f32, kind="ExternalInput")
    out_d = nc.dram_tensor(
        "out", [128, NMACRO, SUB, NHK], f32, kind="ExternalOutput"
    )

    with tile.TileContext(nc) as tc:
        with (
            tc.tile_pool(name="const", bufs=1) as const,
            tc.tile_pool(name="xin", bufs=3) as xin,
            tc.tile_pool(name="zs", bufs=2) as zsp,
            tc.tile_pool(name="sm", bufs=2) as smp,
            tc.tile_pool(name="prod", bufs=3) as prodp,
            tc.tile_pool(name="outp", bufs=4) as outp,
            tc.tile_pool(name="ps_z", bufs=2, space="PSUM") as ps_z,
            tc.tile_pool(name="ps_pw", bufs=1, space="PSUM") as ps_pw,
            tc.tile_pool(name="ps_pt", bufs=1, space="PSUM") as ps_pt,
            tc.tile_pool(name="ps_e", bufs=1, space="PSUM") as ps_e,
        ):
            ident_f = const.tile([128, 128], f32)
            make_identity(nc, ident_f)
            wt = const.tile([128, 5, TRUNK], mmdt)
            nc.gpsimd.dma_start(wt, wt_d[:])
            wr = const.tile([128, 4, PHASES], mmdt)
            nc.gpsimd.dma_start(wr, wr_d[:])
            wh = const.tile([128, 2, NCOL], mmdt)
            nc.gpsimd.dma_start(wh, wh_d[:])
            rb2 = const.tile([PHASES, 1], f32)
            nc.gpsimd.dma_start(rb2, rb2_d[:])
            drb = const.tile([128, NCOL], f32)
            nc.gpsimd.dma_start(drb, drb_d[:])

            # PE prewarm is minimal (ident only): trunk(0) then gates solely
            # on the wt + xt(0) DMAs.  The four pe-slot has_written-setting
            # dummies are emitted inside macro 0, after the router, so they
            # do not put the wh DMA on the critical path of the first
            # matmul.
            pwf = ps_pw.tile([128, MACRO], f32, tag="ppw")
            nc.tensor.transpose(pwf[:7, 0:7], ident_f[:7, :7], ident_f[:7, :7])
            pe_cur = ps_e.tile([128, SUB, MACRO], f32, tag="pe")

            for m in range(NMACRO):
                # ---- load xT tile (host-pre-tiled, fully contiguous) ----
                xt = xin.tile([128, 5, MACRO], mmdt)
                if m == 0:
                    # per-block loads: trunk b=0 starts after 128 KB instead
                    # of waiting for the whole 653 KB tile
                    for b in range(5):
                        nc.sync.dma_start(xt[:, b, :], xt_d[:, m, b])
                else:
                    nc.sync.dma_start(xt, xt_d[:, m])

                petile = pe_cur

                # ---- trunk matmuls: zT [256, 512] over 2 PSUM halves ----
                # (trunk bias rides the constant-1 x row in block 4)
                pz0 = ps_z.tile([128, MACRO], f32, tag="pz")
                pz1 = ps_z.tile([128, MACRO], f32, tag="pz")
                for b in range(5):
                    kb = KBLK[b]
                    nc.tensor.matmul(
                        pz0, wt[:kb, b, 0:128], xt[:kb, b, :],
                        start=(b == 0), stop=(b == 4),
                    )
                    nc.tensor.matmul(
                        pz1, wt[:kb, b, 128:256], xt[:kb, b, :],
                        start=(b == 0), stop=(b == 4),
                    )

                # ---- router matmuls: pwT [7, 512] (h = blocks 0..3) ----
                ppw = ps_pw.tile([128, MACRO], f32, tag="ppw")
                for b in range(4):
                    nc.tensor.matmul(
                        ppw[:PHASES], wr[:128, b, :], xt[:128, b, :],
                        start=(b == 0), stop=(b == 3),
                    )

                if m == 0:
                    # set has_written over the full [128, 320] region of
                    # every pe PSUM slot so the steady-state heads matmuls
                    # can run with start=False and accumulate onto the
                    # ScalarE-prewritten bias
                    for _s in range(SUB):
                        nc.tensor.matmul(
                            petile[:, _s, 0:NCOL], wt[:, 0, 0:128],
                            wh[:, 0, :], start=True, stop=True,
                        )

                # ---- GELU (exact; bias already in pz) -> bf16 z ----
                zs = zsp.tile([128, 2, MACRO], mmdt)
                nc.scalar.activation(
                    out=zs[:, 0, :], in_=pz0, func=AF.Gelu, scale=1.0,
                )
                nc.scalar.activation(
                    out=zs[:, 1, :], in_=pz1, func=AF.Gelu, scale=1.0,
                )
                if m == 0:
                    # bias pre-write for macro 0 (after the gelus in the
                    # ACT queue; overwrites the prewarm garbage, has_written
                    # bits stay set)
                    nc.scalar.copy(
                        out=petile[:, :, 0:NCOL],
                        in_=drb[:, None, :].to_broadcast([128, SUB, NCOL]),
                    )

                # ---- tanh(l/2 + rb/2) phase-major (router bias fused) ----
                thp = smp.tile([PHASES, MACRO], f32, tag="thp")
                nc.scalar.activation(
                    out=thp, in_=ppw[:PHASES], func=AF.Tanh,
                    bias=rb2, scale=0.5,
                )

                # ---- bias pre-write for the NEXT macro's petile (last in
                # the ACT queue so it never delays gelu/tanh) ----
                if m + 1 < NMACRO:
                    pe_cur = ps_e.tile([128, SUB, MACRO], f32, tag="pe")
                    nc.scalar.copy(
                        out=pe_cur[:, :, 0:NCOL],
                        in_=drb[:, None, :].to_broadcast([128, SUB, NCOL]),
                    )

                # ---- heads: petile[:, s, 0:320] per subtile ----
                # The z matmuls run with start=False and accumulate onto the
                # ScalarE-prewritten biases (has_written bits stay set from
                # the prewarm / previous macro, so the PE adds instead of
                # overwriting).
                osb = outp.tile([128, SUB, NHK], f32)
                for s in range(SUB):
                    nc.tensor.matmul(
                        petile[:, s, 0:NCOL], zs[:, 0, ts(s, 128)], wh[:, 0, :],
                        start=False, stop=False,
                    )
                    nc.tensor.matmul(
                        petile[:, s, 0:NCOL], zs[:, 1, ts(s, 128)], wh[:, 1, :],
                        start=False, stop=True,
                    )

                # ---- tanh to token-side (after heads in PE queue) ----
                ppt = ps_pt.tile([128, SUB, PHASES], f32, tag="ppt")
                for s in range(SUB):
                    nc.tensor.transpose(
                        ppt[:, s, :], thp[:, ts(s, 128)],
                        ident_f[:PHASES, :PHASES],
                    )

                # ---- softmax from tanh: exp(l) = (1+t)/(1-t), normalize ----
                den = smp.tile([128, SUB, PHASES], f32, tag="den")
                nc.vector.tensor_scalar(
                    out=den, in0=ppt, scalar1=-1.0, scalar2=1.0,
                    op0=ALU.mult, op1=ALU.add,
                )
                pw8 = smp.tile([128, SUB, NP8], f32, tag="pw8")
                nc.gpsimd.memset(pw8[:, :, PHASES : PHASES + 1], 1.0)
                nc.vector.reciprocal_approx_fast(out=den, in_=den)
                nc.vector.scalar_tensor_tensor(
                    out=pw8[:, :, :PHASES], in0=ppt, scalar=1.0, in1=den,
                    op0=ALU.add, op1=ALU.mult,
                )
                ssum = smp.tile([128, SUB], f32, tag="ssum")
                nc.vector.reduce_sum(
                    out=ssum, in_=pw8[:, :, :PHASES], axis=mybir.AxisListType.X
                )
                rec = smp.tile([128, SUB], f32, tag="rec")
                nc.vector.reciprocal_approx_fast(out=rec, in_=ssum)
                nc.vector.tensor_tensor(
                    out=pw8[:, :, :PHASES],
                    in0=pw8[:, :, :PHASES],
                    in1=rec[:, :, None].to_broadcast([128, SUB, PHASES]),
                    op=ALU.mult,
                )

                # ---- combine: one multiply + one reduce over all 4 banks
                # (for the last macro, per-subtile chunks so the final DVE
                # work and the store drain in a pipeline instead of serially)
                prod = prodp.tile([128, SUB, NHK, NP8], mmdt)
                nchunk = SUB if m == NMACRO - 1 else 1
                cs = SUB // nchunk
                for c in range(nchunk):
                    sl = slice(c * cs, (c + 1) * cs)
                    nc.vector.tensor_tensor(
                        out=prod[:, sl],
                        in0=petile[:, sl, 0:NCOL].rearrange(
                            "p s (hk e) -> p s hk e", e=NP8
                        ),
                        in1=pw8[:, sl, None, :].to_broadcast(
                            [128, cs, NHK, NP8]
                        ),
                        op=ALU.mult,
                    )
                    nc.vector.reduce_sum(
                        out=osb[:, sl], in_=prod[:, sl],
                        axis=mybir.AxisListType.X,
                    )
                    # store on the gpsimd queue so it never head-of-line-
                    # blocks the x loads on the sync queue
                    nc.gpsimd.dma_start(out_d[:, m, sl], osb[:, sl])

    nc.compile()
    return nc


def _host_weights(inp):
    """Rearrange weights into on-device layouts (host-side, one-time)."""
    f = np.float32
    wt = np.zeros((128, 5, TRUNK), f)
    for b in range(4):
        wt[:, b, :] = inp["trunk_w"][b * 128 : (b + 1) * 128]
    wt[:82, 4, :] = inp["trunk_w"][512:IN_DIM]
    wt[82, 4, :] = inp["trunk_b"]  # rides the constant-1 x row

    wr = np.zeros((128, 4, PHASES), f)
    for b in range(4):
        wr[:, b, :] = inp["router_w"][b * 128 : (b + 1) * 128]
    rb2 = np.ascontiguousarray(inp["router_b"].reshape(PHASES, 1)) * 0.5

    # heads: col c = (h*20+k)*8 + p ; p<7 experts, p=7 base
    wh_full = np.zeros((TRUNK, NHK, NP8), f)
    dr_full = np.zeros((NHK, NP8), f)
    wh_full[:, :BINS, :PHASES] = np.transpose(inp["inst_exp_w"], (1, 2, 0))
    wh_full[:, BINS:, :PHASES] = np.transpose(inp["group_exp_w"], (1, 2, 0))
    wh_full[:, :BINS, PHASES] = inp["inst_base_w"]
    wh_full[:, BINS:, PHASES] = inp["group_base_w"]
    dr_full[:BINS, :PHASES] = inp["inst_exp_b"].T
    dr_full[BINS:, :PHASES] = inp["group_exp_b"].T
    dr_full[:BINS, PHASES] = inp["inst_base_b"]
    dr_full[BINS:, PHASES] = inp["group_base_b"]
    wh = wh_full.reshape(TRUNK, NCOL).reshape(2, 128, NCOL).transpose(1, 0, 2).copy()
    drb = np.ascontiguousarray(
        np.broadcast_to(dr_full.reshape(1, NCOL), (128, NCOL))
    )
    return wt, wr, wh, drb, rb2


def kernel(**inputs):
    global LAST_RESULT
    import sys

    if "/opt/trn_rl_repo" not in sys.path:
        sys.path.insert(0, "/opt/trn_rl_repo")
    from concourse.bass_utils import run_bass_kernel_spmd

    inp = {
        k: np.asarray(
            v, dtype=np.float32 if np.asarray(v).dtype != np.int32 else np.int32
        )
        for k, v in inputs.items()
    }

    if "nc" not in _BUILT:
        _BUILT["nc"] = _build_module()
    nc = _BUILT["nc"]

    wt, wr, wh, drb, rb2 = _host_weights(inp)

    x_full = np.concatenate(
        [
            inp["h_t"],
            inp["a_t"],
            inp["d_t"],
            inp["age_embed"],
            np.ones((B, T, 1), np.float32),
        ],
        axis=-1,
    )  # [B, T, 595]; last feature is the constant-1 trunk-bias carrier

    mmdt_name = _mm_dt_name()
    if mmdt_name == "bf16":
        import ml_dtypes

        cdt = ml_dtypes.bfloat16
        x_full = x_full.astype(cdt)
        wt, wr, wh = (a.astype(cdt) for a in (wt, wr, wh))

    in_maps = []
    for c in range(NCORES):
        xc = x_full[c * B_LOC : (c + 1) * B_LOC].reshape(NTOK, IN_DIM + 1)
        # pre-tile to [128, NMACRO, 5, MACRO]: partition p holds feature
        # b*128+p of tokens m*512..m*512+511, 5 KB contiguous per (p, m)
        xpad = np.zeros((640, NTOK), xc.dtype)
        xpad[: IN_DIM + 1] = xc.T
        xtl = np.ascontiguousarray(
            xpad.reshape(5, 128, NMACRO, MACRO).transpose(1, 2, 0, 3)
        )
        in_maps.append(
            {
                "xt": xtl,
                "wt": wt,
                "wr": wr,
                "wh": wh,
                "drb": drb,
                "rb2": rb2,
            }
        )

    res = run_bass_kernel_spmd(nc, in_maps, core_ids=list(range(NCORES)))
    LAST_RESULT = res

    inst = np.empty((B, T, BINS), np.float32)
    grp = np.empty((B, T, BINS), np.float32)
    for c in range(NCORES):
        # device layout [128 p, NMACRO, SUB, NHK] -> token (m, s, p) order
        o = (
            res.results[c]["out"]
            .transpose(1, 2, 0, 3)
            .reshape(B_LOC, T, NHK)
        )
        inst[c * B_LOC : (c + 1) * B_LOC] = o[:, :, :BINS]
        grp[c * B_LOC : (c + 1) * B_LOC] = o[:, :, BINS:]
    return inst, grp
